# revision 23
# baseline (speedup 1.0000x reference)
"""Attention graph convolution (GAT layer) on 8 TRN2 NeuronCores — v3.

Reference computation (all fp32):
    h   = input @ W                      # (N, 64)
    e   = leakyrelu(h@a1 + (h@a2).T)     # (N, N)
    att = softmax(where(adj>0, e, -inf)) # row softmax
    out = elu(att @ h)                   # (N, 64)

Sharding: rows of e/att (= output rows) are split across 8 cores,
no = 1536 rows each.  h (N x 64) is computed on every core (tiny).

Design (v1 476 us -> v2 300 -> v2.1 251 -> this):
  - the adjacency mask is host-baked as an ADDITIVE pre-activation
    offset M[j,i] = 0 (edge) / -150 (no edge), transposed to [j, i] and
    cast bf16.  Masking before the leakyrelu is exact enough:
    exp(lrelu(x-150)) <= e^-24, which is < 1e-10 of any row's softmax
    denominator.  This removes the post-exp mask multiply entirely.
  - a runtime-registered custom DVE op (dve_lrelu_op) fuses the whole
    pre-activation for a chunk into ONE Vector instruction:
        e = max(x, 0.2x),  x = Wh1_i + Wh2_j + M[j,i]
    (replaces tensor_scalar+tensor_scalar+tensor_tensor+mask multiply).
  - a lrelu_act_frac fraction of chunks instead run: tensor_tensor add
    (x+M) then ACT Prelu with the Wh2 bias folded in — balancing DVE
    vs ACT, whose irreducible job is the exp.
  - input.T is host-prepared bf16 and SBUF-resident; each h chunk is a
    single PE matmul; h/Wh2 copies are batched x4 through one PSUM
    tile with 3D-AP copies.
  - everything on-chip is bf16 (fp32 PSUM accumulation): the
    accumulation matmul streams at 1 cycle/row (fp32 is 4).
  - exp runs in x2-chunk sub-batches so the PE's accumulation matmuls
    arrive every ~2 us and HAM keeps the PE at 2.4 GHz.
  - no max-subtraction softmax: |e| < ~30 so U = exp(lrelu(e+M))
    cannot overflow; P.T = h_ext.T @ U with h_ext = [h | 1];
    out = elu(P[:, :64] / P[:, 64]).
"""

import numpy as np

N_TOTAL = 12288
K_IN = 128
F_OUT = 64
N_CORES = 8
ALPHA = 0.2
MASK_NEG = -150.0

_LRELU_OP_NAME = "LRELU_BIAS_MASK_ANT"


def _get_lrelu_op():
    """Runtime-register the fused custom DVE op
        out[p,k] = max(x, x*s1),  x = in0[p,k] + s0[p] + in1[p,k]
    (biased leaky-relu with additive mask) in concourse's custom-DVE
    registry, so the per-NEFF DVE table includes it."""
    import concourse.dve_ops as dve_ops

    if _LRELU_OP_NAME in dve_ops._SUB_OPCODE_FOR_NAME:
        for o in dve_ops.OPS:
            if o.name == _LRELU_OP_NAME:
                return o
        raise RuntimeError(f"{_LRELU_OP_NAME} row registered but op missing")

    from concourse.dve_spec import Spec, Src0, Src1, C0, C1, maxx, lower
    from concourse.dve_spec import _has_src1
    from concourse.dve_uop import DveOpSpec

    x = Src0 + C0 + Src1
    spec = Spec(
        body=maxx(x, x * C1),
        reference=lambda in0, in1, s0, s1, imm2: np.maximum(
            (in0 + s0 + in1), (in0 + s0 + in1) * s1
        ).astype(np.float32),
    )
    shas = {}
    for ver in ("v3", "v4"):
        s = DveOpSpec(name=_LRELU_OP_NAME, opcode=0,
                      uops=lower(spec, ver=ver), rd1_en=_has_src1(spec))
        shas[ver] = s.sha(ver)
    op = dve_ops.DveOp(_LRELU_OP_NAME, spec, subdim=False, uops_sha=shas)
    row = max(dve_ops._SUB_OPCODE_FOR_NAME.values()) + 1
    assert row < 0x20, "no free custom-DVE opcode rows"
    dve_ops.OPS.append(op)
    dve_ops._SUB_OPCODE_FOR_NAME[_LRELU_OP_NAME] = row
    _upgrade_lrelu_op_to_2x(op)
    return op


def _build_lrelu_2x_uop(u1x):
    """Hand-authored 2X_1PORT uop: the 4-stage 1x chain (a = SRC_0+C0;
    x = a+SRC_1; t = x*C1; r = max(x,t)) duplicated for the packed hi
    bf16 halves in stages 4-7.  The lo result is parked in delay lane 0
    at s4; both 16-bit halves of write port 0 are driven."""
    import copy

    from concourse.dve_uop import InpSel, OutSel, OutPath, AluInp, DelayInp
    from concourse.dve_uop import AluOp

    u = copy.deepcopy(u1x)
    u.inp = [InpSel.ZERO, InpSel.SRC_0, InpSel.CONST_0, InpSel.SRC_1,
             InpSel.CONST_1, InpSel.SRC_0_HI, InpSel.SRC_1_HI, InpSel.ZERO]
    u.inp_enable = [0, 1, 1, 1, 1, 1, 1, 0]

    dp = u.datapath_config
    KEEP = DelayInp.PREV_DELAY
    CAP = DelayInp.PREV_ALU_OUT

    def set_stage(stage, op, s0, s1, cap_lane=None):
        d = dp[stage]
        d.op = op
        d.alu_src0 = s0
        d.alu_src1 = s1
        d.alu_out_enable = 1
        d.delay = [KEEP] * 7
        d.delay_enable = [1, 1, 1, 1, 1, 1, 0]
        if cap_lane is not None:
            d.delay[cap_lane] = CAP

    A = AluInp
    # lo chain (delay lanes: 0=SRC_0, 1=C0, 2=SRC_1, 3=C1, 4/5=hi srcs)
    set_stage(0, AluOp.ADD, A.PREV_DELAY_0, A.PREV_DELAY_1)
    set_stage(1, AluOp.ADD, A.PREV_ALU_OUT, A.PREV_DELAY_2)
    set_stage(2, AluOp.MULTIPLY, A.PREV_ALU_OUT, A.PREV_DELAY_3,
              cap_lane=0)              # d0 <- x_lo
    set_stage(3, AluOp.MAX, A.PREV_DELAY_0, A.PREV_ALU_OUT)
    # hi chain
    set_stage(4, AluOp.ADD, A.PREV_DELAY_4, A.PREV_DELAY_1,
              cap_lane=0)              # d0 <- r_lo
    set_stage(5, AluOp.ADD, A.PREV_ALU_OUT, A.PREV_DELAY_5)
    set_stage(6, AluOp.MULTIPLY, A.PREV_ALU_OUT, A.PREV_DELAY_3,
              cap_lane=2)              # d2 <- x_hi
    set_stage(7, AluOp.MAX, A.PREV_DELAY_2, A.PREV_ALU_OUT)

    u.out = {OutPath.WR0_LO: OutSel.DELAY_0, OutPath.WR0_HI: OutSel.ALU_OUT,
             OutPath.WR1_LO: OutSel.ALU_OUT, OutPath.WR1_HI: OutSel.ALU_OUT}
    u.out_enable = {OutPath.WR0_LO: 1, OutPath.WR0_HI: 1,
                    OutPath.WR1_LO: 0, OutPath.WR1_HI: 0}
    return u


def _upgrade_lrelu_op_to_2x(op):
    """Seed concourse's compile cache with a DveOpSpec carrying the 2x
    variant, so the per-NEFF DVE table includes both programs."""
    import concourse.dve_ops as dve_ops
    from concourse.dve_uop import DveOpSpec
    from concourse.dve_spec import lower

    for ver in ("v3",):
        u1x = lower(op.spec, ver=ver)
        u2x = [_build_lrelu_2x_uop(u1x[0])]
        s = DveOpSpec(name=op.name,
                      opcode=dve_ops.get_dve_sub_opcode(op.name),
                      uops=u1x, uops_2x=u2x, perf_max=1, rd1_en=True)
        s.validate(ver)
        dve_ops._COMPILE_CACHE[(op.name, ver)] = s


def build_program(
    nt: int,            # total nodes (j dim)
    no: int,            # nodes owned by this core (i dim)
    batch: int = 4,     # j-chunks per adjacency DMA / phase1b group
    exp_sub: int = 4,   # j-chunks per exp instruction
    lrelu_act_frac: float = 0.0,   # j-chunk fraction with leakyrelu on ACT
    adjt_bufs: int = 3,
    e_bufs: int = 4,
):
    from contextlib import ExitStack

    import concourse.bass as bass
    import concourse.mybir as mybir
    import concourse.tile as tile
    from concourse import bacc
    from concourse.alu_op_type import AluOpType
    from concourse.masks import make_identity

    LRELU_OP = _get_lrelu_op()

    f32 = mybir.dt.float32
    bf16 = mybir.dt.bfloat16
    AF = mybir.ActivationFunctionType

    P = 128
    F = F_OUT
    FE = F + 1                    # h columns + ones column
    K = K_IN
    assert nt % P == 0 and no % P == 0
    ncj = nt // P                 # j chunks (128 rows each)
    nic = no // P                 # i chunks (own rows)
    S = 512                       # i split for matmul free dim / psum banks
    ns = no // S
    assert no % S == 0
    B = batch
    NB = ncj // B
    assert ncj % B == 0 and B % exp_sub == 0
    n_act = int(round(lrelu_act_frac * ncj))
    NW = 4                        # inputT DMA split (column windows)

    nc = bacc.Bacc("TRN2", target_bir_lowering=False, debug=False,
                   num_devices=1)

    inpT = nc.dram_tensor("inputT", [K, nt], bf16, kind="ExternalInput")
    inpT_own = nc.dram_tensor("inputT_own", [K, no], bf16,
                              kind="ExternalInput")
    # additive mask, transposed: maskT[j, i] = 0 if adj[i, j] else -150
    maskT = nc.dram_tensor("maskT", [nt, no], bf16, kind="ExternalInput")
    w_d = nc.dram_tensor("W", [K, F], f32, kind="ExternalInput")
    a_d = nc.dram_tensor("a", [2 * F, 1], f32, kind="ExternalInput")
    out_d = nc.dram_tensor("out", [no, F], f32, kind="ExternalOutput")

    with tile.TileContext(nc) as tc, ExitStack() as ctx:
        consts = ctx.enter_context(tc.tile_pool(name="consts", bufs=1))

        identity = consts.tile([P, P], f32)
        make_identity(nc, identity)

        scr_ps = ctx.enter_context(
            tc.tile_pool(name="scr_ps", bufs=2, space="PSUM"))
        p1b_ps = ctx.enter_context(
            tc.tile_pool(name="p1b_ps", bufs=2, space="PSUM"))

        # ---- phase 0: Wa1 = W @ a1, Wa2 = W @ a2 (f32), cast bf16 ----
        wwa2_f = consts.tile([K, FE], f32)     # [W | Wa2]
        nc.sync.dma_start(wwa2_f[:, 0:F], w_d.ap())
        a_row = consts.tile([1, 2 * F], f32)   # a as a single-partition row
        nc.sync.dma_start(a_row[:], a_d.ap().rearrange("n o -> o n"))
        ito_own = consts.tile([K, no], bf16)   # inputT own window
        nc.sync.dma_start(ito_own[:], inpT_own.ap())

        ones_sb = consts.tile([P, P], f32)
        nc.vector.memset(ones_sb[:], 1.0)
        # replicate a across partitions via a K=1 matmul with a ones row
        a_rep = consts.tile([P, 2 * F], f32)
        a_rep_ps = scr_ps.tile([P, 2 * F], f32, tag="scr")
        nc.tensor.matmul(a_rep_ps[:], ones_sb[0:1, :], a_row[:],
                         start=True, stop=True)
        nc.vector.tensor_copy(a_rep[:], a_rep_ps[:])

        wa12_sb = consts.tile([K, 2], f32)
        wtmp = consts.tile([K, F], f32)
        nc.vector.tensor_tensor(wtmp[:], wwa2_f[:, 0:F], a_rep[:, 0:F],
                                AluOpType.mult)
        nc.vector.tensor_reduce(wa12_sb[:, 0:1], wtmp[:],
                                mybir.AxisListType.X, AluOpType.add)
        nc.vector.tensor_tensor(wtmp[:], wwa2_f[:, 0:F], a_rep[:, F:2 * F],
                                AluOpType.mult)
        nc.vector.tensor_reduce(wa12_sb[:, 1:2], wtmp[:],
                                mybir.AxisListType.X, AluOpType.add)
        nc.vector.tensor_copy(wwa2_f[:, F:FE], wa12_sb[:, 1:2])
        wwa2_bf = consts.tile([K, FE], bf16)   # [W | Wa2] bf16
        nc.vector.tensor_copy(wwa2_bf[:], wwa2_f[:])
        # Wa1 replicated to 128 cols, bf16
        wa1_rep_f = consts.tile([K, P], f32)
        nc.vector.tensor_scalar(wa1_rep_f[:], ones_sb[:], wa12_sb[:, 0:1],
                                None, AluOpType.mult)
        wa1_rep = consts.tile([K, P], bf16)
        nc.vector.tensor_copy(wa1_rep[:], wa1_rep_f[:])

        # ---- wh1_rep[p, i] = Wh1[own i] for all p ------------------------
        wh1_rep = consts.tile([P, no], bf16)
        for s in range(ns):
            w1p = scr_ps.tile([P, S], f32, tag="scr")
            nc.tensor.matmul(w1p[:], wa1_rep[:], ito_own[:, s * S:(s + 1) * S],
                             start=True, stop=True)
            nc.vector.tensor_copy(wh1_rep[:, s * S:(s + 1) * S], w1p[:])

        # ---- adjacency prefetch + exp-table warmup -----------------------
        adjt_pool = ctx.enter_context(tc.tile_pool(name="adjt",
                                                   bufs=adjt_bufs))
        adjt_tiles = {}

        def adjt_fetch(b):
            t = adjt_pool.tile([P, B, no], bf16, tag="adjt")
            nc.sync.dma_start(
                t[:],
                maskT[b * B * P:(b + 1) * B * P, :].rearrange(
                    "(q p) i -> p q i", p=P))
            adjt_tiles[b] = t

        for b in range(min(adjt_bufs - 1, ncj // B)):
            adjt_fetch(b)
        # load the exp table set during the prologue, not at first real exp
        nc.scalar.activation(wtmp[:, 0:1], wa12_sb[:, 0:1], AF.Exp)

        # ---- inputT resident in SBUF (windowed DMA) ----------------------
        ito_sb = consts.tile([K, nt], bf16)
        WCOL = nt // NW
        for w in range(NW):
            nc.sync.dma_start(ito_sb[:, w * WCOL:(w + 1) * WCOL],
                              inpT[:, w * WCOL:(w + 1) * WCOL])

        # ---- phase 1b: h_ext[:, jc, :] = [h | 1], wh2 --------------------
        h_ext = consts.tile([P, ncj, FE], bf16)
        wh2_sb = consts.tile([P, ncj], f32)
        nc.vector.memset(h_ext[:, :, F], 1.0)

        def phase1b_group(b):
            # B chunks' h/Wh2 through one PSUM tile, two 3D-AP copies
            hw_ps = p1b_ps.tile([P, B, FE], f32, tag="p1b")
            for q in range(B):
                jc = b * B + q
                nc.tensor.matmul(hw_ps[:, q, :], ito_sb[:, jc * P:(jc + 1) * P],
                                 wwa2_bf[:], start=True, stop=True)
            nc.vector.tensor_copy(h_ext[:, b * B:(b + 1) * B, 0:F],
                                  hw_ps[:, :, 0:F])
            nc.vector.tensor_copy(wh2_sb[:, b * B:(b + 1) * B],
                                  hw_ps[:, :, F])

        def act_path(jc):
            return (jc * 7919) % ncj < n_act

        # ---- phase 2: main loop over j batches ---------------------------
        pt_pool = ctx.enter_context(
            tc.tile_pool(name="pt_acc", bufs=1, space="PSUM"))
        pt_ps = pt_pool.tile([FE, no], f32)
        # HAM pacer: a [K=128, M=1, N=1] matmul fires as each chunk's
        # lrelu completes, so the PE never accumulates a ~3.4 us idle
        # window and stays at 2.4 GHz between accumulation bursts.
        pace_pool = ctx.enter_context(
            tc.tile_pool(name="pace_ps", bufs=1, space="PSUM"))

        # phase-1b production runs PH1B_AHEAD batches ahead of use, so a
        # batch's wh2 copies never sit behind the previous batch's
        # accumulation matmuls in the PE/ACT FIFOs (the wh2 -> lrelu ->
        # exp -> matmul chain would otherwise serialize per batch).
        PH1B_AHEAD = 2
        for b in range(min(PH1B_AHEAD, NB)):
            phase1b_group(b)

        mm_pending = []

        def emit_mms(b, e_sb):
            for q in range(B):
                jc = b * B + q
                for s in range(ns):
                    nc.tensor.matmul(pt_ps[:, s * S:(s + 1) * S],
                                     h_ext[:, jc, :],
                                     e_sb[:, q, s * S:(s + 1) * S],
                                     start=(jc == 0),
                                     stop=(jc == ncj - 1))
        with (
            tc.tile_pool(name="epool", bufs=e_bufs) as e_pool,
        ):
            for b in range(NB):
                if b not in adjt_tiles:
                    adjt_fetch(b)
                adjt = adjt_tiles.pop(b)
                nf = b + adjt_bufs - 1
                if nf < NB and nf not in adjt_tiles:
                    adjt_fetch(nf)
                if b + PH1B_AHEAD < NB:
                    phase1b_group(b + PH1B_AHEAD)
                e_sb = e_pool.tile([P, B, no], bf16, tag="e")
                for qs in range(B // exp_sub):
                    for q in range(qs * exp_sub, (qs + 1) * exp_sub):
                        jc = b * B + q
                        if act_path(jc):
                            # x+M on DVE, then lrelu with Wh2 bias on ACT
                            nc.vector.tensor_tensor(
                                e_sb[:, q, :], wh1_rep[:], adjt[:, q, :],
                                AluOpType.add)
                            nc.scalar.activation(
                                e_sb[:, q, :], e_sb[:, q, :], AF.Prelu,
                                bias=wh2_sb[:, jc:jc + 1],
                                scale=1.0, alpha=ALPHA)
                        else:
                            # one fused DVE op: max(x, 0.2x),
                            # x = Wh1 + Wh2 + M (2X_1PORT packed bf16)
                            ci = nc.vector._custom_dve(
                                LRELU_OP, out=e_sb[:, q, :], in0=wh1_rep[:],
                                in1=adjt[:, q, :],
                                s0=wh2_sb[:, jc:jc + 1], s1=ALPHA)
                            ci.ins.perf_max = 1
                        pace = pace_pool.tile([1, 1], f32, tag="pace")
                        nc.tensor.matmul(pace[:], wh1_rep[:, 0:1],
                                         e_sb[:, q, 0:1],
                                         start=True, stop=True)
                    nc.scalar.activation(
                        e_sb[:, qs * exp_sub:(qs + 1) * exp_sub, :],
                        e_sb[:, qs * exp_sub:(qs + 1) * exp_sub, :], AF.Exp)
                # accumulation matmuls are emitted one batch LATE so the
                # PE always has a fully-ready group queued: the inter-
                # group wait drops under HAM's ~3.4 us idle window and
                # the PE stays at 2.4 GHz.
                mm_pending.append((b, e_sb))
                if len(mm_pending) > 1:
                    emit_mms(*mm_pending.pop(0))
            while mm_pending:
                emit_mms(*mm_pending.pop(0))

        # ---- phase 3: out = elu(P[:, :64] / P[:, 64]) --------------------
        pt_sb = consts.tile([FE, no], f32)
        with tc.tile_pool(name="fin_sb", bufs=4) as fin_sb:
            for ic in range(nic):
                nc.vector.tensor_copy(pt_sb[:, ic * P:(ic + 1) * P],
                                      pt_ps[:, ic * P:(ic + 1) * P])
                ptp = scr_ps.tile([P, FE], f32, tag="scr")
                nc.tensor.transpose(ptp[:], pt_sb[:, ic * P:(ic + 1) * P],
                                    identity[0:FE, 0:FE])
                rec = fin_sb.tile([P, 1], f32, tag="rec")
                nc.vector.reciprocal(rec[:], ptp[:, F:FE])
                hp = fin_sb.tile([P, F], f32, tag="hp")
                nc.vector.tensor_scalar(hp[:], ptp[:, 0:F], rec[:], None,
                                        AluOpType.mult)
                # elu(x) = max(x,0) + exp(min(x,0)) - 1
                mn = fin_sb.tile([P, F], f32, tag="mn")
                nc.vector.tensor_scalar(mn[:], hp[:], 0.0, None, AluOpType.min)
                nc.scalar.activation(mn[:], mn[:], AF.Exp)
                nc.vector.tensor_scalar(hp[:], hp[:], 0.0, None, AluOpType.max)
                ob = fin_sb.tile([P, F], f32, tag="ob")
                nc.vector.scalar_tensor_tensor(
                    ob[:], mn[:], 1.0, hp[:],
                    AluOpType.subtract, AluOpType.add)
                nc.sync.dma_start(out_d[ic * P:(ic + 1) * P, :], ob[:])

    nc.compile()
    return nc


_CACHE = {}


def _get_program(nt, no, **kw):
    key = (nt, no, tuple(sorted(kw.items())))
    if key not in _CACHE:
        _CACHE[key] = build_program(nt, no, **kw)
    return _CACHE[key]


def prepare(inputs, **kw):
    """Build (program, per-core input maps) from full unsharded inputs."""
    import ml_dtypes

    bf16 = ml_dtypes.bfloat16
    input = np.ascontiguousarray(inputs["input"], dtype=np.float32)
    adj = inputs["adj"]
    W = np.ascontiguousarray(inputs["W"], dtype=np.float32)
    a = np.ascontiguousarray(inputs["a"], dtype=np.float32)

    nt = input.shape[0]
    no = nt // N_CORES
    nc = _get_program(nt, no, **kw)

    inputT = input.T.astype(bf16, order="C")
    in_maps = []
    for c in range(N_CORES):
        blk = adj[c * no:(c + 1) * no].T  # [nt, no] view
        m = np.where(blk != 0, np.float32(0.0),
                     np.float32(MASK_NEG)).astype(bf16, order="C")
        in_maps.append({
            "inputT": inputT,
            "inputT_own": np.ascontiguousarray(
                inputT[:, c * no:(c + 1) * no]),
            "maskT": m,
            "W": W,
            "a": a,
        })
    return nc, in_maps


def kernel(input, adj, W, a):
    from concourse.bass_utils import run_bass_kernel_spmd

    nc, in_maps = prepare({"input": input, "adj": adj, "W": W, "a": a})
    res = run_bass_kernel_spmd(nc, in_maps, list(range(N_CORES)))
    return np.concatenate([r["out"] for r in res.results], axis=0)


# revision 24
# speedup vs baseline: 1.0250x; 1.0250x over previous
"""Attention graph convolution (GAT layer) on 8 TRN2 NeuronCores — v3.

Reference computation (all fp32):
    h   = input @ W                      # (N, 64)
    e   = leakyrelu(h@a1 + (h@a2).T)     # (N, N)
    att = softmax(where(adj>0, e, -inf)) # row softmax
    out = elu(att @ h)                   # (N, 64)

Sharding: rows of e/att (= output rows) are split across 8 cores,
no = 1536 rows each.  h (N x 64) is computed on every core (tiny).

Design (v1 476 us -> v2 300 -> v2.1 251 -> this):
  - the adjacency mask is host-baked as an ADDITIVE pre-activation
    offset M[j,i] = 0 (edge) / -150 (no edge), transposed to [j, i] and
    cast bf16.  Masking before the leakyrelu is exact enough:
    exp(lrelu(x-150)) <= e^-24, which is < 1e-10 of any row's softmax
    denominator.  This removes the post-exp mask multiply entirely.
  - a runtime-registered custom DVE op (dve_lrelu_op) fuses the whole
    pre-activation for a chunk into ONE Vector instruction:
        e = max(x, 0.2x),  x = Wh1_i + Wh2_j + M[j,i]
    (replaces tensor_scalar+tensor_scalar+tensor_tensor+mask multiply).
  - a lrelu_act_frac fraction of chunks instead run: tensor_tensor add
    (x+M) then ACT Prelu with the Wh2 bias folded in — balancing DVE
    vs ACT, whose irreducible job is the exp.
  - input.T is host-prepared bf16 and SBUF-resident; each h chunk is a
    single PE matmul; h/Wh2 copies are batched x4 through one PSUM
    tile with 3D-AP copies.
  - everything on-chip is bf16 (fp32 PSUM accumulation): the
    accumulation matmul streams at 1 cycle/row (fp32 is 4).
  - exp runs in x2-chunk sub-batches so the PE's accumulation matmuls
    arrive every ~2 us and HAM keeps the PE at 2.4 GHz.
  - no max-subtraction softmax: |e| < ~30 so U = exp(lrelu(e+M))
    cannot overflow; P.T = h_ext.T @ U with h_ext = [h | 1];
    out = elu(P[:, :64] / P[:, 64]).
"""

import numpy as np

N_TOTAL = 12288
K_IN = 128
F_OUT = 64
N_CORES = 8
ALPHA = 0.2
MASK_NEG = -150.0

_LRELU_OP_NAME = "LRELU_BIAS_MASK_ANT"


def _get_lrelu_op():
    """Runtime-register the fused custom DVE op
        out[p,k] = max(x, x*s1),  x = in0[p,k] + s0[p] + in1[p,k]
    (biased leaky-relu with additive mask) in concourse's custom-DVE
    registry, so the per-NEFF DVE table includes it."""
    import concourse.dve_ops as dve_ops

    if _LRELU_OP_NAME in dve_ops._SUB_OPCODE_FOR_NAME:
        for o in dve_ops.OPS:
            if o.name == _LRELU_OP_NAME:
                return o
        raise RuntimeError(f"{_LRELU_OP_NAME} row registered but op missing")

    from concourse.dve_spec import Spec, Src0, Src1, C0, C1, maxx, lower
    from concourse.dve_spec import _has_src1
    from concourse.dve_uop import DveOpSpec

    x = Src0 + C0 + Src1
    spec = Spec(
        body=maxx(x, x * C1),
        reference=lambda in0, in1, s0, s1, imm2: np.maximum(
            (in0 + s0 + in1), (in0 + s0 + in1) * s1
        ).astype(np.float32),
    )
    shas = {}
    for ver in ("v3", "v4"):
        s = DveOpSpec(name=_LRELU_OP_NAME, opcode=0,
                      uops=lower(spec, ver=ver), rd1_en=_has_src1(spec))
        shas[ver] = s.sha(ver)
    op = dve_ops.DveOp(_LRELU_OP_NAME, spec, subdim=False, uops_sha=shas)
    row = max(dve_ops._SUB_OPCODE_FOR_NAME.values()) + 1
    assert row < 0x20, "no free custom-DVE opcode rows"
    dve_ops.OPS.append(op)
    dve_ops._SUB_OPCODE_FOR_NAME[_LRELU_OP_NAME] = row
    _upgrade_lrelu_op_to_2x(op)
    return op


def _build_lrelu_2x_uop(u1x):
    """Hand-authored 2X_1PORT uop: the 4-stage 1x chain (a = SRC_0+C0;
    x = a+SRC_1; t = x*C1; r = max(x,t)) duplicated for the packed hi
    bf16 halves in stages 4-7.  The lo result is parked in delay lane 0
    at s4; both 16-bit halves of write port 0 are driven."""
    import copy

    from concourse.dve_uop import InpSel, OutSel, OutPath, AluInp, DelayInp
    from concourse.dve_uop import AluOp

    u = copy.deepcopy(u1x)
    u.inp = [InpSel.ZERO, InpSel.SRC_0, InpSel.CONST_0, InpSel.SRC_1,
             InpSel.CONST_1, InpSel.SRC_0_HI, InpSel.SRC_1_HI, InpSel.ZERO]
    u.inp_enable = [0, 1, 1, 1, 1, 1, 1, 0]

    dp = u.datapath_config
    KEEP = DelayInp.PREV_DELAY
    CAP = DelayInp.PREV_ALU_OUT

    def set_stage(stage, op, s0, s1, cap_lane=None):
        d = dp[stage]
        d.op = op
        d.alu_src0 = s0
        d.alu_src1 = s1
        d.alu_out_enable = 1
        d.delay = [KEEP] * 7
        d.delay_enable = [1, 1, 1, 1, 1, 1, 0]
        if cap_lane is not None:
            d.delay[cap_lane] = CAP

    A = AluInp
    # lo chain (delay lanes: 0=SRC_0, 1=C0, 2=SRC_1, 3=C1, 4/5=hi srcs)
    set_stage(0, AluOp.ADD, A.PREV_DELAY_0, A.PREV_DELAY_1)
    set_stage(1, AluOp.ADD, A.PREV_ALU_OUT, A.PREV_DELAY_2)
    set_stage(2, AluOp.MULTIPLY, A.PREV_ALU_OUT, A.PREV_DELAY_3,
              cap_lane=0)              # d0 <- x_lo
    set_stage(3, AluOp.MAX, A.PREV_DELAY_0, A.PREV_ALU_OUT)
    # hi chain
    set_stage(4, AluOp.ADD, A.PREV_DELAY_4, A.PREV_DELAY_1,
              cap_lane=0)              # d0 <- r_lo
    set_stage(5, AluOp.ADD, A.PREV_ALU_OUT, A.PREV_DELAY_5)
    set_stage(6, AluOp.MULTIPLY, A.PREV_ALU_OUT, A.PREV_DELAY_3,
              cap_lane=2)              # d2 <- x_hi
    set_stage(7, AluOp.MAX, A.PREV_DELAY_2, A.PREV_ALU_OUT)

    u.out = {OutPath.WR0_LO: OutSel.DELAY_0, OutPath.WR0_HI: OutSel.ALU_OUT,
             OutPath.WR1_LO: OutSel.ALU_OUT, OutPath.WR1_HI: OutSel.ALU_OUT}
    u.out_enable = {OutPath.WR0_LO: 1, OutPath.WR0_HI: 1,
                    OutPath.WR1_LO: 0, OutPath.WR1_HI: 0}
    return u


def _upgrade_lrelu_op_to_2x(op):
    """Seed concourse's compile cache with a DveOpSpec carrying the 2x
    variant, so the per-NEFF DVE table includes both programs."""
    import concourse.dve_ops as dve_ops
    from concourse.dve_uop import DveOpSpec
    from concourse.dve_spec import lower

    for ver in ("v3",):
        u1x = lower(op.spec, ver=ver)
        u2x = [_build_lrelu_2x_uop(u1x[0])]
        s = DveOpSpec(name=op.name,
                      opcode=dve_ops.get_dve_sub_opcode(op.name),
                      uops=u1x, uops_2x=u2x, perf_max=1, rd1_en=True)
        s.validate(ver)
        dve_ops._COMPILE_CACHE[(op.name, ver)] = s


def build_program(
    nt: int,            # total nodes (j dim)
    no: int,            # nodes owned by this core (i dim)
    batch: int = 4,     # j-chunks per adjacency DMA / phase1b group
    exp_sub: int = 4,   # j-chunks per exp instruction
    lrelu_act_frac: float = 0.0,   # j-chunk fraction with leakyrelu on ACT
    adjt_bufs: int = 3,
    e_bufs: int = 4,
):
    from contextlib import ExitStack

    import concourse.bass as bass
    import concourse.mybir as mybir
    import concourse.tile as tile
    from concourse import bacc
    from concourse.alu_op_type import AluOpType
    from concourse.masks import make_identity

    LRELU_OP = _get_lrelu_op()

    f32 = mybir.dt.float32
    bf16 = mybir.dt.bfloat16
    AF = mybir.ActivationFunctionType

    P = 128
    F = F_OUT
    FE = F + 1                    # h columns + ones column
    K = K_IN
    assert nt % P == 0 and no % P == 0
    ncj = nt // P                 # j chunks (128 rows each)
    nic = no // P                 # i chunks (own rows)
    S = 512                       # i split for matmul free dim / psum banks
    ns = no // S
    assert no % S == 0
    B = batch
    NB = ncj // B
    assert ncj % B == 0 and B % exp_sub == 0
    n_act = int(round(lrelu_act_frac * ncj))
    NW = 4                        # inputT DMA split (column windows)

    nc = bacc.Bacc("TRN2", target_bir_lowering=False, debug=False,
                   num_devices=1)

    inpT = nc.dram_tensor("inputT", [K, nt], bf16, kind="ExternalInput")
    inpT_own = nc.dram_tensor("inputT_own", [K, no], bf16,
                              kind="ExternalInput")
    # additive mask, transposed: maskT[j, i] = 0 if adj[i, j] else -150
    maskT = nc.dram_tensor("maskT", [nt, no], bf16, kind="ExternalInput")
    w_d = nc.dram_tensor("W", [K, F], f32, kind="ExternalInput")
    a_d = nc.dram_tensor("a", [2 * F, 1], f32, kind="ExternalInput")
    out_d = nc.dram_tensor("out", [no, F], f32, kind="ExternalOutput")

    with tile.TileContext(nc) as tc, ExitStack() as ctx:
        consts = ctx.enter_context(tc.tile_pool(name="consts", bufs=1))

        identity = consts.tile([P, P], f32)
        make_identity(nc, identity)

        scr_ps = ctx.enter_context(
            tc.tile_pool(name="scr_ps", bufs=2, space="PSUM"))
        p1b_ps = ctx.enter_context(
            tc.tile_pool(name="p1b_ps", bufs=2, space="PSUM"))

        # ---- phase 0: Wa1 = W @ a1, Wa2 = W @ a2 (f32), cast bf16 ----
        wwa2_f = consts.tile([K, FE], f32)     # [W | Wa2]
        nc.sync.dma_start(wwa2_f[:, 0:F], w_d.ap())
        a_row = consts.tile([1, 2 * F], f32)   # a as a single-partition row
        nc.sync.dma_start(a_row[:], a_d.ap().rearrange("n o -> o n"))
        ito_own = consts.tile([K, no], bf16)   # inputT own window
        nc.sync.dma_start(ito_own[:], inpT_own.ap())

        ones_sb = consts.tile([P, P], f32)
        nc.vector.memset(ones_sb[:], 1.0)
        # replicate a across partitions via a K=1 matmul with a ones row
        a_rep = consts.tile([P, 2 * F], f32)
        a_rep_ps = scr_ps.tile([P, 2 * F], f32, tag="scr")
        nc.tensor.matmul(a_rep_ps[:], ones_sb[0:1, :], a_row[:],
                         start=True, stop=True)
        nc.vector.tensor_copy(a_rep[:], a_rep_ps[:])

        wa12_sb = consts.tile([K, 2], f32)
        wtmp = consts.tile([K, F], f32)
        nc.vector.tensor_tensor(wtmp[:], wwa2_f[:, 0:F], a_rep[:, 0:F],
                                AluOpType.mult)
        nc.vector.tensor_reduce(wa12_sb[:, 0:1], wtmp[:],
                                mybir.AxisListType.X, AluOpType.add)
        nc.vector.tensor_tensor(wtmp[:], wwa2_f[:, 0:F], a_rep[:, F:2 * F],
                                AluOpType.mult)
        nc.vector.tensor_reduce(wa12_sb[:, 1:2], wtmp[:],
                                mybir.AxisListType.X, AluOpType.add)
        nc.vector.tensor_copy(wwa2_f[:, F:FE], wa12_sb[:, 1:2])
        wwa2_bf = consts.tile([K, FE], bf16)   # [W | Wa2] bf16
        nc.vector.tensor_copy(wwa2_bf[:], wwa2_f[:])
        # Wa1 replicated to 128 cols, bf16
        wa1_rep_f = consts.tile([K, P], f32)
        nc.vector.tensor_scalar(wa1_rep_f[:], ones_sb[:], wa12_sb[:, 0:1],
                                None, AluOpType.mult)
        wa1_rep = consts.tile([K, P], bf16)
        nc.vector.tensor_copy(wa1_rep[:], wa1_rep_f[:])

        # ---- wh1_rep[p, i] = Wh1[own i] for all p ------------------------
        wh1_rep = consts.tile([P, no], bf16)
        for s in range(ns):
            w1p = scr_ps.tile([P, S], f32, tag="scr")
            nc.tensor.matmul(w1p[:], wa1_rep[:], ito_own[:, s * S:(s + 1) * S],
                             start=True, stop=True)
            nc.vector.tensor_copy(wh1_rep[:, s * S:(s + 1) * S], w1p[:])

        # ---- adjacency prefetch + exp-table warmup -----------------------
        adjt_pool = ctx.enter_context(tc.tile_pool(name="adjt",
                                                   bufs=adjt_bufs))
        adjt_tiles = {}

        def adjt_fetch(b):
            t = adjt_pool.tile([P, B, no], bf16, tag="adjt")
            nc.sync.dma_start(
                t[:],
                maskT[b * B * P:(b + 1) * B * P, :].rearrange(
                    "(q p) i -> p q i", p=P))
            adjt_tiles[b] = t

        for b in range(min(adjt_bufs - 1, ncj // B)):
            adjt_fetch(b)
        # load the exp table set during the prologue, not at first real exp
        nc.scalar.activation(wtmp[:, 0:1], wa12_sb[:, 0:1], AF.Exp)

        # ---- inputT resident in SBUF (windowed DMA) ----------------------
        ito_sb = consts.tile([K, nt], bf16)
        WCOL = nt // NW
        for w in range(NW):
            nc.sync.dma_start(ito_sb[:, w * WCOL:(w + 1) * WCOL],
                              inpT[:, w * WCOL:(w + 1) * WCOL])

        # ---- phase 1b: h_ext[:, jc, :] = [h | 1], wh2 --------------------
        h_ext = consts.tile([P, ncj, FE], bf16)
        wh2_sb = consts.tile([P, ncj], f32)
        nc.vector.memset(h_ext[:, :, F], 1.0)

        def phase1b_group(b):
            # B chunks' h/Wh2 through one PSUM tile, two 3D-AP copies
            hw_ps = p1b_ps.tile([P, B, FE], f32, tag="p1b")
            for q in range(B):
                jc = b * B + q
                nc.tensor.matmul(hw_ps[:, q, :], ito_sb[:, jc * P:(jc + 1) * P],
                                 wwa2_bf[:], start=True, stop=True)
            nc.vector.tensor_copy(h_ext[:, b * B:(b + 1) * B, 0:F],
                                  hw_ps[:, :, 0:F])
            nc.vector.tensor_copy(wh2_sb[:, b * B:(b + 1) * B],
                                  hw_ps[:, :, F])

        def act_path(jc):
            return (jc * 7919) % ncj < n_act

        # ---- phase 2: main loop over j batches ---------------------------
        pt_pool = ctx.enter_context(
            tc.tile_pool(name="pt_acc", bufs=1, space="PSUM"))
        pt_ps = pt_pool.tile([FE, no], f32)
        # HAM pacer: a [K=128, M=1, N=1] matmul fires as each chunk's
        # lrelu completes, so the PE never accumulates a ~3.4 us idle
        # window and stays at 2.4 GHz between accumulation bursts.
        pace_pool = ctx.enter_context(
            tc.tile_pool(name="pace_ps", bufs=1, space="PSUM"))

        # phase-1b production runs PH1B_AHEAD batches ahead of use, so a
        # batch's wh2 copies never sit behind the previous batch's
        # accumulation matmuls in the PE/ACT FIFOs (the wh2 -> lrelu ->
        # exp -> matmul chain would otherwise serialize per batch).
        PH1B_AHEAD = 2
        for b in range(min(PH1B_AHEAD, NB)):
            phase1b_group(b)

        mm_pending = []

        def emit_mms(b, e_sb):
            for q in range(B):
                jc = b * B + q
                for s in range(ns):
                    nc.tensor.matmul(pt_ps[:, s * S:(s + 1) * S],
                                     h_ext[:, jc, :],
                                     e_sb[:, q, s * S:(s + 1) * S],
                                     start=(jc == 0),
                                     stop=(jc == ncj - 1))
        with (
            tc.tile_pool(name="epool", bufs=e_bufs) as e_pool,
        ):
            for b in range(NB):
                if b not in adjt_tiles:
                    adjt_fetch(b)
                adjt = adjt_tiles.pop(b)
                nf = b + adjt_bufs - 1
                if nf < NB and nf not in adjt_tiles:
                    adjt_fetch(nf)
                if b + PH1B_AHEAD < NB:
                    phase1b_group(b + PH1B_AHEAD)
                e_sb = e_pool.tile([P, B, no], bf16, tag="e")
                for qs in range(B // exp_sub):
                    for q in range(qs * exp_sub, (qs + 1) * exp_sub):
                        jc = b * B + q
                        if act_path(jc):
                            # x+M on DVE, then lrelu with Wh2 bias on ACT
                            nc.vector.tensor_tensor(
                                e_sb[:, q, :], wh1_rep[:], adjt[:, q, :],
                                AluOpType.add)
                            nc.scalar.activation(
                                e_sb[:, q, :], e_sb[:, q, :], AF.Prelu,
                                bias=wh2_sb[:, jc:jc + 1],
                                scale=1.0, alpha=ALPHA)
                        else:
                            # one fused DVE op: max(x, 0.2x),
                            # x = Wh1 + Wh2 + M (2X_1PORT packed bf16)
                            ci = nc.vector._custom_dve(
                                LRELU_OP, out=e_sb[:, q, :], in0=wh1_rep[:],
                                in1=adjt[:, q, :],
                                s0=wh2_sb[:, jc:jc + 1], s1=ALPHA)
                            ci.ins.perf_max = 1
                        pace = pace_pool.tile([1, 1], f32, tag="pace")
                        nc.tensor.matmul(pace[:], wh1_rep[:, 0:1],
                                         e_sb[:, q, 0:1],
                                         start=True, stop=True)
                    nc.scalar.activation(
                        e_sb[:, qs * exp_sub:(qs + 1) * exp_sub, :],
                        e_sb[:, qs * exp_sub:(qs + 1) * exp_sub, :], AF.Exp)
                # accumulation matmuls are emitted one batch LATE so the
                # PE always has a fully-ready group queued: the inter-
                # group wait drops under HAM's ~3.4 us idle window and
                # the PE stays at 2.4 GHz.
                # emit accumulation matmuls in PAIRS of batches: a 24-MM
                # run is long enough (~10 us cold) for HAM to un-throttle
                # the PE mid-burst, instead of paying the cold clock on
                # every 12-MM group.
                mm_pending.append((b, e_sb))
                if len(mm_pending) >= 3:
                    emit_mms(*mm_pending.pop(0))
                    emit_mms(*mm_pending.pop(0))
            while mm_pending:
                emit_mms(*mm_pending.pop(0))

        # ---- phase 3: out = elu(P[:, :64] / P[:, 64]) --------------------
        pt_sb = consts.tile([FE, no], f32)
        with tc.tile_pool(name="fin_sb", bufs=4) as fin_sb:
            for ic in range(nic):
                nc.vector.tensor_copy(pt_sb[:, ic * P:(ic + 1) * P],
                                      pt_ps[:, ic * P:(ic + 1) * P])
                ptp = scr_ps.tile([P, FE], f32, tag="scr")
                nc.tensor.transpose(ptp[:], pt_sb[:, ic * P:(ic + 1) * P],
                                    identity[0:FE, 0:FE])
                rec = fin_sb.tile([P, 1], f32, tag="rec")
                nc.vector.reciprocal(rec[:], ptp[:, F:FE])
                hp = fin_sb.tile([P, F], f32, tag="hp")
                nc.vector.tensor_scalar(hp[:], ptp[:, 0:F], rec[:], None,
                                        AluOpType.mult)
                # elu(x) = max(x,0) + exp(min(x,0)) - 1
                mn = fin_sb.tile([P, F], f32, tag="mn")
                nc.vector.tensor_scalar(mn[:], hp[:], 0.0, None, AluOpType.min)
                nc.scalar.activation(mn[:], mn[:], AF.Exp)
                nc.vector.tensor_scalar(hp[:], hp[:], 0.0, None, AluOpType.max)
                ob = fin_sb.tile([P, F], f32, tag="ob")
                nc.vector.scalar_tensor_tensor(
                    ob[:], mn[:], 1.0, hp[:],
                    AluOpType.subtract, AluOpType.add)
                nc.sync.dma_start(out_d[ic * P:(ic + 1) * P, :], ob[:])

    nc.compile()
    return nc


_CACHE = {}


def _get_program(nt, no, **kw):
    key = (nt, no, tuple(sorted(kw.items())))
    if key not in _CACHE:
        _CACHE[key] = build_program(nt, no, **kw)
    return _CACHE[key]


def prepare(inputs, **kw):
    """Build (program, per-core input maps) from full unsharded inputs."""
    import ml_dtypes

    bf16 = ml_dtypes.bfloat16
    input = np.ascontiguousarray(inputs["input"], dtype=np.float32)
    adj = inputs["adj"]
    W = np.ascontiguousarray(inputs["W"], dtype=np.float32)
    a = np.ascontiguousarray(inputs["a"], dtype=np.float32)

    nt = input.shape[0]
    no = nt // N_CORES
    nc = _get_program(nt, no, **kw)

    inputT = input.T.astype(bf16, order="C")
    in_maps = []
    for c in range(N_CORES):
        blk = adj[c * no:(c + 1) * no].T  # [nt, no] view
        m = np.where(blk != 0, np.float32(0.0),
                     np.float32(MASK_NEG)).astype(bf16, order="C")
        in_maps.append({
            "inputT": inputT,
            "inputT_own": np.ascontiguousarray(
                inputT[:, c * no:(c + 1) * no]),
            "maskT": m,
            "W": W,
            "a": a,
        })
    return nc, in_maps


def kernel(input, adj, W, a):
    from concourse.bass_utils import run_bass_kernel_spmd

    nc, in_maps = prepare({"input": input, "adj": adj, "W": W, "a": a})
    res = run_bass_kernel_spmd(nc, in_maps, list(range(N_CORES)))
    return np.concatenate([r["out"] for r in res.results], axis=0)
